# revision 17
# baseline (speedup 1.0000x reference)
"""Dynamic GQA attention (nn_DGQA) on 8 Trainium2 NeuronCores.

Strategy
--------
The "dynamic" part of DGQA (the query-head -> kv-head assignment, derived
from k-projection norms) is computed on the host in numpy and folded into a
host-side gather of the Wk/Wv weight columns.  The device program is a
static, uniform SPMD kernel: plain 8-head attention per core.

Sharding: core c handles batch b = c//2 and half of the query heads
(half = c%2).  Each core computes a partial output projection (contraction
over its 512 head-dims of Wp); the host sums the two partials per batch and
adds the bias.

v2 improvements over the first working version:
  * kv dedup: query heads are host-sorted by kv head so duplicated kv heads
    become adjacent pairs; the k/v projection chains compute only the unique
    kv heads ("slots"), and the PV stationary (vplus) is shared per pair.
    The program is built per sharing pattern (cached on the pattern).
  * softmax reciprocal off ScalarE: the per-unit sums row [1, 1024] is
    DMA-reshaped to [128, 8], inverted with the 128-lane-parallel DVE
    reciprocal, cast to bf16 and DMA'd back; ScalarE now runs only the exp
    stream.
  * the two per-unit reciprocal-broadcast matmuls are column-packed into a
    single PSUM bank (concurrent col tiles (0,0) / (0,64)).
  * output is written bf16 (host upcasts and sums partials) halving the
    output DMA volume; input/output DMAs are spread over several engine
    queues; xt loads are split into column pieces so the first projection
    chains start earlier.

Device kernel (per core, all matmuls bf16, fp32 accumulation):
  qT = wq.T @ xt            (wq pre-scaled by D^-0.5 on host)
  kT = wk_u.T @ xt          (unique kv slots only)
  v  = xt.T @ wv_u          (written into per-slot "vplus" tiles with a
                             ones-column per slot)
  per head pair (row-tiled QK, d=64 contraction, 2 heads concurrently):
    scoresT[key, qp] in PSUM -> exp on ScalarE -> bf16 SBUF
    PV: outT[d, qp] (+ sums row from the ones-column) accumulated over key
        chunks with the pair-shared vplus as the stationary operand
  normalization: DMA/DVE reciprocal, broadcast with col-packed k=1 matmuls,
  multiplied in on VectorE
  proj: out[row, :] partial = outT.T @ wp  (contraction over this core's
  douts), written bf16
"""

import numpy as np
import ml_dtypes

B, P, DIM, H, HKV = 4, 2048, 1024, 16, 8
D = DIM // H          # 64
NCORES = 8
HPC = H // 2          # query heads per core = 8
NPAIR = HPC // 2      # head pairs per core = 4
DPC = HPC * D         # head-dims per core = 512
VW = D + 1            # v columns per slot incl. ones column = 65

BF16 = ml_dtypes.bfloat16


# ----------------------------------------------------------------- host math

def _ratios_np(k_bhpd: np.ndarray, cache: np.ndarray) -> np.ndarray:
    """Numpy replica of the reference's _ratios (fp32, round-half-even)."""
    mags = np.sqrt((k_bhpd * k_bhpd).sum(axis=(2, 3))).sum(axis=0)
    diff = np.abs(cache - mags)
    r = np.round(diff / diff.sum() * H).astype(np.int64)
    while r.sum() > H:
        r[np.argmax(r)] -= 1
    while r.sum() < H:
        r[np.argmin(r)] += 1
    return r


def _kv_id(x: np.ndarray, Wk: np.ndarray, cache: np.ndarray) -> np.ndarray:
    k = (x.reshape(B * P, DIM).astype(np.float32) @ Wk.astype(np.float32))
    k = k.reshape(B, P, HKV, D).transpose(0, 2, 1, 3)
    r = _ratios_np(k, cache.astype(np.float32))
    return np.searchsorted(np.cumsum(r), np.arange(H), side="right")


def _plan(kv_id: np.ndarray):
    """Head ordering + kv slot plan shared by the program and the in_maps.

    Returns (F, perms) where F is the per-pair "heads share a kv head" flag
    tuple (ANDed over both halves so the SPMD program is uniform), and
    perms[half] is the local head order (sorted by kv id).
    """
    perms = []
    shared = []
    for half in range(2):
        kv = kv_id[half * HPC:(half + 1) * HPC]
        perm = sorted(range(HPC), key=lambda i: (kv[i], i))
        perms.append(perm)
        shared.append([kv[perm[2 * j]] == kv[perm[2 * j + 1]]
                       for j in range(NPAIR)])
    F = tuple(bool(shared[0][j] and shared[1][j]) for j in range(NPAIR))
    return F, perms


def _slot_map(F):
    """slot_of[local_head] and per-slot kt copy destinations [(pair, half)]."""
    slot_of = [0] * HPC
    dests = []
    for j in range(NPAIR):
        if F[j]:
            dests.append([(j, 0), (j, 1)])
            slot_of[2 * j] = slot_of[2 * j + 1] = len(dests) - 1
        else:
            dests.append([(j, 0)])
            slot_of[2 * j] = len(dests) - 1
            dests.append([(j, 1)])
            slot_of[2 * j + 1] = len(dests) - 1
    S = len(dests)
    S2 = S + (S % 2)
    return slot_of, dests, S, S2


# ----------------------------------------------------- walrus wait splitting

def _split_wide_waits(nc, max_waits=1):
    """This toolchain's walrus allows only one sync-wait per instruction;
    move extra waits onto preceding NOPs on the same engine."""
    import bass_rust
    import concourse.mybir as mybir

    n = 0
    for f in nc.m.functions:
        for blk in f.blocks:
            out = []
            changed = False
            for ins in blk.instructions:
                si = ins.sync_info
                if si is not None and si.on_wait is not None and \
                        len(si.on_wait) > max_waits:
                    waits = list(si.on_wait)
                    keep = waits[-max_waits:]
                    extra = waits[:-max_waits]
                    for j in range(0, len(extra), max_waits):
                        n += 1
                        nop = mybir.InstNoOp(
                            name=f"waitsplit-{n}", ins=[], outs=[])
                        nop.engine = ins.engine
                        nop.sync_info = bass_rust.SyncInfo(
                            on_wait=extra[j:j + max_waits], on_update=[])
                        out.append(nop)
                    ins.sync_info = bass_rust.SyncInfo(
                        on_wait=keep, on_update=list(si.on_update or []))
                    changed = True
                out.append(ins)
            if changed:
                blk.instructions = out
    return n


# ------------------------------------------------------------ device program

def build_program(F, p_len: int = P, split_waits: bool = True):
    """Build the SPMD Bass/Tile program for sharing pattern F."""
    from contextlib import ExitStack

    import concourse.bass as bass
    import concourse.tile as tile
    from concourse import mybir

    F32 = mybir.dt.float32
    BF = mybir.dt.bfloat16
    EXP = mybir.ActivationFunctionType.Exp

    slot_of, kt_dests, S, S2 = _slot_map(F)
    KVW = 64 * S2                 # wk_u / wv_u column count (padded even)

    NKC = p_len // 128            # key chunks
    NQT = max(p_len // 512, 1)    # qp tiles of width...
    QW = min(512, p_len)          # qp tile width
    NRM = p_len // 128            # output row chunks
    RPT = NRM // NQT              # row chunks per qp tile
    NDIN = DIM // 128             # contraction chunks for projections

    nc = bass.Bass("TRN2", target_bir_lowering=False, debug=False,
                   num_devices=NCORES)
    xt_d = nc.dram_tensor("xt", [DIM, p_len], BF, kind="ExternalInput").ap()
    wq_d = nc.dram_tensor("wq", [DIM, DPC], BF, kind="ExternalInput").ap()
    wk_d = nc.dram_tensor("wk", [DIM, KVW], BF, kind="ExternalInput").ap()
    wv_d = nc.dram_tensor("wv", [DIM, KVW], BF, kind="ExternalInput").ap()
    wp_d = nc.dram_tensor("wp", [DPC, DIM], BF, kind="ExternalInput").ap()
    out_d = nc.dram_tensor("out", [p_len, DIM], BF, kind="ExternalOutput").ap()

    with tile.TileContext(nc) as tc, ExitStack() as ctx:
        sbw = ctx.enter_context(tc.tile_pool(name="sbw", bufs=1))
        sbx = ctx.enter_context(tc.tile_pool(name="sbx", bufs=1))
        sbqk = ctx.enter_context(tc.tile_pool(name="sbqk", bufs=1))
        sbeg = ctx.enter_context(tc.tile_pool(name="sbeg", bufs=6))
        sbot = ctx.enter_context(tc.tile_pool(name="sbot", bufs=2))
        sbo = ctx.enter_context(tc.tile_pool(name="sbo", bufs=3))
        sbr = ctx.enter_context(tc.tile_pool(name="sbr", bufs=2))
        psb = ctx.enter_context(tc.tile_pool(name="psb", bufs=2, space="PSUM"))
        pssg = ctx.enter_context(tc.tile_pool(name="pssg", bufs=2, space="PSUM"))
        pspv = ctx.enter_context(tc.tile_pool(name="pspv", bufs=2, space="PSUM"))

        # ---- input loads -------------------------------------------------
        # Spread input DMAs across three otherwise-idle engine queues and
        # load xt in column pieces so the first projection chains (and the
        # first attention unit, which only needs qp tile 0) start as soon as
        # possible.
        wq_sb = [sbw.tile([128, DPC], BF, tag=f"wq{i}", name=f"wq{i}") for i in range(NDIN)]
        wk_sb = [sbw.tile([128, KVW], BF, tag=f"wk{i}", name=f"wk{i}") for i in range(NDIN)]
        wv_sb = [sbw.tile([128, KVW], BF, tag=f"wv{i}", name=f"wv{i}") for i in range(NDIN)]
        xt_sb = [sbx.tile([128, p_len], BF, tag=f"xt{i}", name=f"xt{i}") for i in range(NDIN)]
        wp_sb = [sbw.tile([128, DIM], BF, tag=f"wp{i}", name=f"wp{i}") for i in range(DPC // 128)]

        # sync + gpsimd + scalar queues; scalar only carries startup loads
        # (the exp stream hasn't started yet).
        # Touch Ln and Exp immediately (before anything else lands in the
        # ScalarE queue) so the ACT table set containing both loads at kernel
        # start; the tail finalize uses Ln.
        warm = sbw.tile([1, 8], F32, tag="warm", name="warm")
        nc.vector.memset(warm[:], 1.0)
        nc.scalar.activation(warm[:], warm[:], mybir.ActivationFunctionType.Ln)
        nc.scalar.activation(warm[:], warm[:], EXP)

        # sync + gpsimd queues only: scalar must stay clear for the exp
        # stream (DMA issue costs ~0.6us each and would delay the first exp).
        dma_engines = [nc.sync, nc.gpsimd]
        dma_rr = [0]

        def dma_in(dst, src):
            eng = dma_engines[dma_rr[0] % len(dma_engines)]
            dma_rr[0] += 1
            eng.dma_start(dst, src)

        # qp-tile-0 inputs first: wq, wk and the first xt column piece.
        for i in range(NDIN):
            dma_in(wq_sb[i][:], wq_d[128 * i:128 * (i + 1), :])
            dma_in(wk_sb[i][:], wk_d[128 * i:128 * (i + 1), :])
            dma_in(xt_sb[i][:, 0:512], xt_d[128 * i:128 * (i + 1), 0:512])
        for i in range(NDIN):
            dma_in(wv_sb[i][:], wv_d[128 * i:128 * (i + 1), :])
            dma_in(xt_sb[i][:, 512:1024], xt_d[128 * i:128 * (i + 1), 512:1024])
        for t5 in range(2, p_len // 512):
            for i in range(NDIN):
                dma_in(xt_sb[i][:, 512 * t5:512 * (t5 + 1)],
                       xt_d[128 * i:128 * (i + 1), 512 * t5:512 * (t5 + 1)])
        for i in range(DPC // 128):
            dma_in(wp_sb[i][:], wp_d[128 * i:128 * (i + 1), :])

        ones_sb = sbw.tile([1, 64], BF, tag="ones", name="ones")
        nc.vector.memset(ones_sb[:], 1.0)

        LAG = 3  # PV chunks behind QK in the modulo schedule
        # ---- stage B: projection chain emitters ---------------------------
        # qT: [DPC, p_len] as row-of-128 tiles (2 heads per tile)
        # kT: per-pair tiles [128, p_len]; both halves of a shared pair are
        #     copied from the same unique-slot chain output.
        qt_sb = [sbqk.tile([128, p_len], BF, tag=f"qt{m}", name=f"qt{m}") for m in range(NPAIR)]
        kt_sb = [sbqk.tile([128, p_len], BF, tag=f"kt{m}", name=f"kt{m}") for m in range(NPAIR)]
        vplus_sb = sbqk.tile([128, NKC * S * VW], BF, tag="vplus", name="vplus")
        vp3 = vplus_sb[:].rearrange("p (kc s w) -> p kc s w", kc=NKC, s=S)

        def vplus(kc, s):
            off = (kc * S + s) * VW
            return vplus_sb[:, off:off + VW]

        nc.vector.memset(vp3[:, :, :, D:VW], 1.0)
        NT5 = p_len // 512

        def qk_chain_gen(dst, w_sb, m, t):
            ps = psb.tile([128, 512], F32, tag="psb", name="psb")
            for kd in range(NDIN):
                nc.tensor.matmul(
                    ps[:], w_sb[kd][:, 128 * m:128 * (m + 1)],
                    xt_sb[kd][:, 512 * t:512 * (t + 1)],
                    start=(kd == 0), stop=(kd == NDIN - 1))
                if kd % 2 == 1 and kd < NDIN - 1:
                    yield
            nc.vector.tensor_copy(dst[m][:, 512 * t:512 * (t + 1)], ps[:])

        def kt_chain_gen(mu, t):
            # unique-slot k chain: psum rows 0:64 = slot 2mu, 64:128 = 2mu+1
            ps = psb.tile([128, 512], F32, tag="psb", name="psb")
            for kd in range(NDIN):
                nc.tensor.matmul(
                    ps[:], wk_sb[kd][:, 128 * mu:128 * (mu + 1)],
                    xt_sb[kd][:, 512 * t:512 * (t + 1)],
                    start=(kd == 0), stop=(kd == NDIN - 1))
                if kd % 2 == 1 and kd < NDIN - 1:
                    yield
            for hf in range(2):
                s = 2 * mu + hf
                if s >= S:
                    break
                for (j, jh) in kt_dests[s]:
                    nc.vector.tensor_copy(
                        kt_sb[j][64 * jh:64 * (jh + 1), 512 * t:512 * (t + 1)],
                        ps[64 * hf:64 * (hf + 1), :])

        def v_chain_gen(rm):
            ps = psb.tile([128, 512], F32, tag="psb", name="psb")
            for kd in range(NDIN):
                nc.tensor.matmul(
                    ps[:, 0:KVW], xt_sb[kd][:, 128 * rm:128 * (rm + 1)],
                    wv_sb[kd][:],
                    start=(kd == 0), stop=(kd == NDIN - 1))
                if kd % 2 == 1 and kd < NDIN - 1:
                    yield
            nc.vector.tensor_copy(
                vp3[:, rm, :, 0:D],
                ps[:, 0:64 * S].rearrange("p (s d) -> p s d", s=S))

        def proj_gen(t, rj, ot_tiles):
            o_sb = sbo.tile([128, DIM], BF, tag="osb", name="osb")
            for e2 in range(DIM // 512):
                ps = psb.tile([128, 512], F32, tag="psb", name="psb")
                for pair in range(NPAIR):
                    nc.tensor.matmul(
                        ps[:],
                        ot_tiles[pair][:, 128 * rj:128 * (rj + 1)],
                        wp_sb[pair][:, 512 * e2:512 * (e2 + 1)],
                        start=(pair == 0), stop=(pair == NPAIR - 1))
                    if pair == 1:
                        yield
                nc.vector.tensor_copy(o_sb[:, 512 * e2:512 * (e2 + 1)], ps[:])
                yield
            row0 = (t * RPT + rj) * 128
            # split every store across both queues so neither backlogs and
            # the final drain is short; the norm DMAs on gpsimd are tiny and
            # jump the 128KB-granular queue quickly.
            nc.sync.dma_start(out_d[row0:row0 + 64, :], o_sb[0:64, :])
            nc.gpsimd.dma_start(out_d[row0 + 64:row0 + 128, :],
                                o_sb[64:128, :])

        import heapq

        total_chunks = NQT * NPAIR * NKC
        pump_q = []   # (deadline, seq, earliest, gen)
        pump_seq = [0]

        def add_gen(deadline, earliest, gen):
            pump_seq[0] += 1
            heapq.heappush(pump_q, (deadline, pump_seq[0], earliest, gen))

        # unit 0 needs qt/kt (pair 0, t 0) before its first instruction is
        # emitted: run those chains inline (exhaust the generators).
        for _ in qk_chain_gen(qt_sb, wq_sb, 0, 0):
            pass
        for _ in kt_chain_gen(0, 0):
            pass

        for rm in range(NKC):
            add_gen(max(rm + 1, 0), 0, v_chain_gen(rm))
        for t in range(NT5):
            for m in range(NPAIR):
                dl = max((NT5 * t + m) * NKC - 2, 0)
                if not (m == 0 and t == 0):
                    add_gen(dl, 0, qk_chain_gen(qt_sb, wq_sb, m, t))
                if m < S2 // 2 and not (m == 0 and t == 0):
                    add_gen(dl, 0, kt_chain_gen(m, t))

        pump_state = {"gen": None, "dl": 0}

        def pump(g, budget=1):
            steps = 0
            while True:
                if pump_state["gen"] is None:
                    if not pump_q or pump_q[0][2] > g:
                        return
                    dl, _, _, gen = heapq.heappop(pump_q)
                    pump_state["gen"] = gen
                    pump_state["dl"] = dl
                urgent = pump_state["dl"] <= g + 2
                if steps >= budget and not urgent:
                    return
                try:
                    next(pump_state["gen"])
                    steps += 1
                except StopIteration:
                    pump_state["gen"] = None

        # ---- stage C + D: attention then projection -----------------------
        # Flat modulo-schedule over units=(t, pair): PV lags QK by LAG chunks
        # (crossing unit boundaries) so the ScalarE exp stream never starves
        # and PE never waits on the normalization chain.
        units = [(t, pair) for t in range(NQT) for pair in range(NPAIR)]

        class Unit:
            pass

        def start_unit(i):
            u = Unit()
            u.t, u.pair = units[i]
            u.qt, u.kt = qt_sb[u.pair], kt_sb[u.pair]
            u.s0, u.s1 = slot_of[2 * u.pair], slot_of[2 * u.pair + 1]
            u.pv0 = pspv.tile([128, QW], F32, tag="pv", name="pv")
            u.pv1 = pspv.tile([128, QW], F32, tag="pv", name="pv")
            u.egs = [None] * NKC
            return u

        def qk_exp(u, kc):
            sg = pssg.tile([128, 2 * QW], F32, tag="sg", name="sg")
            nc.tensor.matmul(
                sg[:, 0:QW], u.kt[0:64, 128 * kc:128 * (kc + 1)],
                u.qt[0:64, QW * u.t:QW * (u.t + 1)], start=True, stop=True)
            nc.tensor.matmul(
                sg[:, QW:2 * QW], u.kt[64:128, 128 * kc:128 * (kc + 1)],
                u.qt[64:128, QW * u.t:QW * (u.t + 1)], start=True, stop=True)
            eg = sbeg.tile([128, 2 * QW], BF, tag="eg", name="eg")
            nc.scalar.activation(eg[:], sg[:], EXP)
            u.egs[kc] = eg

        def pv_mm(u, kc):
            nc.tensor.matmul(
                u.pv0[0:VW, :], vplus(kc, u.s0), u.egs[kc][:, 0:QW],
                start=(kc == 0), stop=(kc == NKC - 1))
            nc.tensor.matmul(
                u.pv1[0:VW, :], vplus(kc, u.s1), u.egs[kc][:, QW:2 * QW],
                start=(kc == 0), stop=(kc == NKC - 1))
            u.egs[kc] = None

        def finalize_a1(u):
            # copy unnormalized outT (both heads into one [128, QW] tile) and
            # the raw sums rows to SBUF so the two pv psum banks recycle
            # quickly, and DMA-reshape the sums to [128, 8].
            u.s = sbr.tile([128, QW], F32, tag="s", name="s")
            nc.vector.tensor_copy(u.s[0:64, :], u.pv0[0:D, :])
            nc.vector.tensor_copy(u.s[64:128, :], u.pv1[0:D, :])
            sm = sbr.tile([1, 2 * QW], F32, tag="sm", name="sm")
            nc.vector.tensor_copy(sm[:, 0:QW], u.pv0[D:VW, :])
            nc.vector.tensor_copy(sm[:, QW:2 * QW], u.pv1[D:VW, :])
            u.smt = sbr.tile([128, 8], F32, tag="smt", name="smt")
            nc.gpsimd.dma_start(u.smt[:], sm[:])

        def finalize_a2(u):
            # a few chunks later (so the DVE doesn't head-of-line block on
            # the DMA): invert on DVE (exact, 128-lane-parallel), cast bf16,
            # DMA back into [1, 1024] layout for the broadcast matmuls.
            smr = sbr.tile([128, 8], F32, tag="smr", name="smr")
            nc.vector.reciprocal(smr[:], u.smt[:])
            smb = sbr.tile([128, 8], BF, tag="smb", name="smb")
            nc.vector.tensor_copy(smb[:], smr[:])
            u.rc = sbr.tile([1, 2 * QW], BF, tag="rc", name="rc")
            nc.gpsimd.dma_start(u.rc[:], smb[:])

        def finalize_b(u, ot_tiles):
            # broadcast reciprocals across partitions with two col-packed
            # (concurrent) k=1 matmuls into one PSUM bank, then scale.
            rb = psb.tile([128, QW], F32, tag="psb", name="psb")
            nc.tensor.matmul(rb[0:64, :], ones_sb[:], u.rc[:, 0:QW],
                             start=True, stop=True)
            nc.tensor.matmul(rb[64:128, :], ones_sb[:], u.rc[:, QW:2 * QW],
                             start=True, stop=True)
            ot = sbot.tile([128, QW], BF, tag=f"ot{u.pair}", name=f"ot{u.pair}")
            nc.vector.tensor_mul(ot[:], u.s[:], rb[:])
            ot_tiles[u.pair] = ot

        ot_by_t = {t: [None] * NPAIR for t in range(NQT)}
        FA2_KC = LAG + 3
        FB_KC = min(LAG + 6, NKC - 1)

        def step_prev(u, kc, g):
            # deferred post-processing of the previous unit, spread across
            # this unit's chunk stream
            if kc >= LAG and not u.fa_done:
                finalize_a1(u)
                u.fa_done = True
            elif kc >= FA2_KC and not u.fa2_done:
                finalize_a2(u)
                u.fa2_done = True
            elif kc >= FB_KC and not u.fb_done:
                finalize_b(u, ot_by_t[u.t])
                u.fb_done = True
                if u.pair == NPAIR - 1:
                    for rj in range(RPT):
                        add_gen(total_chunks + u.t, g + 1,
                                proj_gen(u.t, rj, ot_by_t[u.t]))

        def finalize_ln(u):
            # tail path (final unit): no pipeline slack for the DMA round
            # trip, but ScalarE is idle once the exp stream ends — compute
            # the reciprocal there as exp(-ln(sums)); Ln/Exp share the
            # already-loaded table set.
            u.s = sbr.tile([128, QW], F32, tag="s", name="s")
            nc.vector.tensor_copy(u.s[0:64, :], u.pv0[0:D, :])
            nc.vector.tensor_copy(u.s[64:128, :], u.pv1[0:D, :])
            sm = sbr.tile([1, 2 * QW], F32, tag="sm", name="sm")
            nc.vector.tensor_copy(sm[:, 0:QW], u.pv0[D:VW, :])
            nc.vector.tensor_copy(sm[:, QW:2 * QW], u.pv1[D:VW, :])
            lg = sbr.tile([1, 2 * QW], F32, tag="lg", name="lg")
            nc.scalar.activation(lg[:], sm[:], mybir.ActivationFunctionType.Ln)
            u.rc = sbr.tile([1, 2 * QW], BF, tag="rc", name="rc")
            nc.scalar.activation(u.rc[:], lg[:], EXP, scale=-1.0)

        def drain_prev(u, g):
            if not u.fa_done:
                finalize_ln(u)
            elif not u.fa2_done:
                finalize_a2(u)
            if not u.fb_done:
                finalize_b(u, ot_by_t[u.t])
                if u.pair == NPAIR - 1:
                    for rj in range(RPT):
                        add_gen(total_chunks + u.t, 0,
                                proj_gen(u.t, rj, ot_by_t[u.t]))

        prev = None
        cur = start_unit(0)
        for i in range(len(units)):
            cur.fa_done = cur.fa2_done = cur.fb_done = False
            for kc in range(NKC):
                g = i * NKC + kc
                qk_exp(cur, kc)
                pump(g)
                gk = kc - LAG
                if gk >= 0:
                    pv_mm(cur, gk)
                elif prev is not None:
                    pv_mm(prev, NKC + gk)
                if prev is not None:
                    step_prev(prev, kc, g)
            if prev is not None and not prev.fb_done:
                drain_prev(prev, i * NKC + NKC - 1)
            prev, cur = cur, (start_unit(i + 1) if i + 1 < len(units) else None)
        for gk in range(NKC - LAG, NKC):
            pv_mm(prev, gk)
            pump(total_chunks)
        drain_prev(prev, total_chunks)
        g = total_chunks
        while pump_q or pump_state["gen"] is not None:
            pump(g, budget=100)
            g += 1

    if split_waits:
        _split_wide_waits(nc, max_waits=1)
    return nc


_PROGRAMS = {}


def _get_program(F):
    if F not in _PROGRAMS:
        _PROGRAMS[F] = build_program(F, P)
    return _PROGRAMS[F]


# ------------------------------------------------------------------- kernel

def make_in_maps(x, Wq, Wk, Wv, Wp, bp, cache):
    x = np.asarray(x, np.float32)
    Wq = np.asarray(Wq, np.float32)
    Wk = np.asarray(Wk, np.float32)
    Wv = np.asarray(Wv, np.float32)
    Wp = np.asarray(Wp, np.float32)
    kv_id = _kv_id(x, Wk, np.asarray(cache, np.float32))
    F, perms = _plan(kv_id)
    slot_of, kt_dests, S, S2 = _slot_map(F)

    scale = 1.0 / np.sqrt(D)
    in_maps = []
    xt_b = [np.ascontiguousarray(x[b].T).astype(BF16) for b in range(B)]
    for c in range(NCORES):
        b, half = divmod(c, 2)
        perm = perms[half]
        heads = [half * HPC + p for p in perm]
        # slot s is represented by (pair j, half jh) = kt_dests[s][0]
        slot_kv = [kv_id[heads[2 * j + jh]] for (j, jh) in
                   (kt_dests[s][0] for s in range(S))]
        wk_c = np.zeros((DIM, 64 * S2), np.float32)
        wv_c = np.zeros((DIM, 64 * S2), np.float32)
        for s, kv in enumerate(slot_kv):
            wk_c[:, 64 * s:64 * (s + 1)] = Wk[:, kv * D:(kv + 1) * D]
            wv_c[:, 64 * s:64 * (s + 1)] = Wv[:, kv * D:(kv + 1) * D]
        wq_c = np.concatenate(
            [Wq[:, h * D:(h + 1) * D] for h in heads], axis=1) * scale
        wp_c = np.concatenate(
            [Wp[h * D:(h + 1) * D, :] for h in heads], axis=0)
        in_maps.append({
            "xt": xt_b[b],
            "wq": np.ascontiguousarray(wq_c).astype(BF16),
            "wk": np.ascontiguousarray(wk_c).astype(BF16),
            "wv": np.ascontiguousarray(wv_c).astype(BF16),
            "wp": np.ascontiguousarray(wp_c).astype(BF16),
        })
    return F, in_maps


_WARMED = set()


def kernel(x, Wq, Wk, Wv, Wp, bp, cache, _trace=False):
    from concourse.bass_utils import run_bass_kernel_spmd

    F, in_maps = make_in_maps(x, Wq, Wk, Wv, Wp, bp, cache)
    nc = _get_program(F)
    if F not in _WARMED:
        # First execution on a cold NEFF has been observed racing the ACT
        # table load; run once and discard.
        run_bass_kernel_spmd(nc, in_maps, core_ids=list(range(NCORES)),
                             trace=False)
        _WARMED.add(F)
    res = run_bass_kernel_spmd(nc, in_maps, core_ids=list(range(NCORES)),
                               trace=_trace)
    bp32 = np.asarray(bp, np.float32)
    out = np.empty((B, P, DIM), np.float32)
    for b in range(B):
        out[b] = (res.results[2 * b]["out"].astype(np.float32)
                  + res.results[2 * b + 1]["out"].astype(np.float32) + bp32)
    if _trace:
        kernel.last_exec_time_ns = res.exec_time_ns
    return out


# revision 19
# speedup vs baseline: 1.0062x; 1.0062x over previous
"""Dynamic GQA attention (nn_DGQA) on 8 Trainium2 NeuronCores.

Strategy
--------
The "dynamic" part of DGQA (the query-head -> kv-head assignment, derived
from k-projection norms) is computed on the host in numpy and folded into a
host-side gather of the Wk/Wv weight columns.  The device program is a
static, uniform SPMD kernel: plain 8-head attention per core.

Sharding: core c handles batch b = c//2 and half of the query heads
(half = c%2).  Each core computes a partial output projection (contraction
over its 512 head-dims of Wp); the host sums the two partials per batch and
adds the bias.

v2 improvements over the first working version:
  * kv dedup: query heads are host-sorted by kv head so duplicated kv heads
    become adjacent pairs; the k/v projection chains compute only the unique
    kv heads ("slots"), and the PV stationary (vplus) is shared per pair.
    The program is built per sharing pattern (cached on the pattern).
  * softmax reciprocal off ScalarE: the per-unit sums row [1, 1024] is
    DMA-reshaped to [128, 8], inverted with the 128-lane-parallel DVE
    reciprocal, cast to bf16 and DMA'd back; ScalarE now runs only the exp
    stream.
  * the two per-unit reciprocal-broadcast matmuls are column-packed into a
    single PSUM bank (concurrent col tiles (0,0) / (0,64)).
  * output is written bf16 (host upcasts and sums partials) halving the
    output DMA volume; input/output DMAs are spread over several engine
    queues; xt loads are split into column pieces so the first projection
    chains start earlier.

Device kernel (per core, all matmuls bf16, fp32 accumulation):
  qT = wq.T @ xt            (wq pre-scaled by D^-0.5 on host)
  kT = wk_u.T @ xt          (unique kv slots only)
  v  = xt.T @ wv_u          (written into per-slot "vplus" tiles with a
                             ones-column per slot)
  per head pair (row-tiled QK, d=64 contraction, 2 heads concurrently):
    scoresT[key, qp] in PSUM -> exp on ScalarE -> bf16 SBUF
    PV: outT[d, qp] (+ sums row from the ones-column) accumulated over key
        chunks with the pair-shared vplus as the stationary operand
  normalization: DMA/DVE reciprocal, broadcast with col-packed k=1 matmuls,
  multiplied in on VectorE
  proj: out[row, :] partial = outT.T @ wp  (contraction over this core's
  douts), written bf16
"""

import numpy as np
import ml_dtypes

B, P, DIM, H, HKV = 4, 2048, 1024, 16, 8
D = DIM // H          # 64
NCORES = 8
HPC = H // 2          # query heads per core = 8
NPAIR = HPC // 2      # head pairs per core = 4
DPC = HPC * D         # head-dims per core = 512
VW = D + 1            # v columns per slot incl. ones column = 65

BF16 = ml_dtypes.bfloat16


# ----------------------------------------------------------------- host math

def _ratios_np(k_bhpd: np.ndarray, cache: np.ndarray) -> np.ndarray:
    """Numpy replica of the reference's _ratios (fp32, round-half-even)."""
    mags = np.sqrt((k_bhpd * k_bhpd).sum(axis=(2, 3))).sum(axis=0)
    diff = np.abs(cache - mags)
    r = np.round(diff / diff.sum() * H).astype(np.int64)
    while r.sum() > H:
        r[np.argmax(r)] -= 1
    while r.sum() < H:
        r[np.argmin(r)] += 1
    return r


def _kv_id(x: np.ndarray, Wk: np.ndarray, cache: np.ndarray) -> np.ndarray:
    k = (x.reshape(B * P, DIM).astype(np.float32) @ Wk.astype(np.float32))
    k = k.reshape(B, P, HKV, D).transpose(0, 2, 1, 3)
    r = _ratios_np(k, cache.astype(np.float32))
    return np.searchsorted(np.cumsum(r), np.arange(H), side="right")


def _plan(kv_id: np.ndarray):
    """Head ordering + kv slot plan shared by the program and the in_maps.

    Returns (F, perms) where F is the per-pair "heads share a kv head" flag
    tuple (ANDed over both halves so the SPMD program is uniform), and
    perms[half] is the local head order (sorted by kv id).
    """
    perms = []
    shared = []
    for half in range(2):
        kv = kv_id[half * HPC:(half + 1) * HPC]
        perm = sorted(range(HPC), key=lambda i: (kv[i], i))
        perms.append(perm)
        shared.append([kv[perm[2 * j]] == kv[perm[2 * j + 1]]
                       for j in range(NPAIR)])
    F = tuple(bool(shared[0][j] and shared[1][j]) for j in range(NPAIR))
    return F, perms


def _slot_map(F):
    """slot_of[local_head] and per-slot kt copy destinations [(pair, half)]."""
    slot_of = [0] * HPC
    dests = []
    for j in range(NPAIR):
        if F[j]:
            dests.append([(j, 0), (j, 1)])
            slot_of[2 * j] = slot_of[2 * j + 1] = len(dests) - 1
        else:
            dests.append([(j, 0)])
            slot_of[2 * j] = len(dests) - 1
            dests.append([(j, 1)])
            slot_of[2 * j + 1] = len(dests) - 1
    S = len(dests)
    S2 = S + (S % 2)
    return slot_of, dests, S, S2


# ----------------------------------------------------- walrus wait splitting

def _split_wide_waits(nc, max_waits=1):
    """This toolchain's walrus allows only one sync-wait per instruction;
    move extra waits onto preceding NOPs on the same engine."""
    import bass_rust
    import concourse.mybir as mybir

    n = 0
    for f in nc.m.functions:
        for blk in f.blocks:
            out = []
            changed = False
            for ins in blk.instructions:
                si = ins.sync_info
                if si is not None and si.on_wait is not None and \
                        len(si.on_wait) > max_waits:
                    waits = list(si.on_wait)
                    keep = waits[-max_waits:]
                    extra = waits[:-max_waits]
                    for j in range(0, len(extra), max_waits):
                        n += 1
                        nop = mybir.InstNoOp(
                            name=f"waitsplit-{n}", ins=[], outs=[])
                        nop.engine = ins.engine
                        nop.sync_info = bass_rust.SyncInfo(
                            on_wait=extra[j:j + max_waits], on_update=[])
                        out.append(nop)
                    ins.sync_info = bass_rust.SyncInfo(
                        on_wait=keep, on_update=list(si.on_update or []))
                    changed = True
                out.append(ins)
            if changed:
                blk.instructions = out
    return n


# ------------------------------------------------------------ device program

def build_program(F, p_len: int = P, split_waits: bool = True):
    """Build the SPMD Bass/Tile program for sharing pattern F."""
    from contextlib import ExitStack

    import concourse.bass as bass
    import concourse.tile as tile
    from concourse import mybir

    F32 = mybir.dt.float32
    BF = mybir.dt.bfloat16
    EXP = mybir.ActivationFunctionType.Exp

    slot_of, kt_dests, S, S2 = _slot_map(F)
    KVW = 64 * S2                 # wk_u / wv_u column count (padded even)

    NKC = p_len // 128            # key chunks
    NQT = max(p_len // 512, 1)    # qp tiles of width...
    QW = min(512, p_len)          # qp tile width
    NRM = p_len // 128            # output row chunks
    RPT = NRM // NQT              # row chunks per qp tile
    NDIN = DIM // 128             # contraction chunks for projections

    nc = bass.Bass("TRN2", target_bir_lowering=False, debug=False,
                   num_devices=NCORES)
    xt_d = nc.dram_tensor("xt", [DIM, p_len], BF, kind="ExternalInput").ap()
    wq_d = nc.dram_tensor("wq", [DIM, DPC], BF, kind="ExternalInput").ap()
    wk_d = nc.dram_tensor("wk", [DIM, KVW], BF, kind="ExternalInput").ap()
    wv_d = nc.dram_tensor("wv", [DIM, KVW], BF, kind="ExternalInput").ap()
    wp_d = nc.dram_tensor("wp", [DPC, DIM], BF, kind="ExternalInput").ap()
    out_d = nc.dram_tensor("out", [p_len, DIM], BF, kind="ExternalOutput").ap()

    with tile.TileContext(nc) as tc, ExitStack() as ctx:
        sbw = ctx.enter_context(tc.tile_pool(name="sbw", bufs=1))
        sbx = ctx.enter_context(tc.tile_pool(name="sbx", bufs=1))
        sbqk = ctx.enter_context(tc.tile_pool(name="sbqk", bufs=1))
        sbeg = ctx.enter_context(tc.tile_pool(name="sbeg", bufs=6))
        sbot = ctx.enter_context(tc.tile_pool(name="sbot", bufs=2))
        sbo = ctx.enter_context(tc.tile_pool(name="sbo", bufs=3))
        sbr = ctx.enter_context(tc.tile_pool(name="sbr", bufs=2))
        psb = ctx.enter_context(tc.tile_pool(name="psb", bufs=2, space="PSUM"))
        pssg = ctx.enter_context(tc.tile_pool(name="pssg", bufs=2, space="PSUM"))
        pspv = ctx.enter_context(tc.tile_pool(name="pspv", bufs=2, space="PSUM"))

        # ---- input loads -------------------------------------------------
        # Spread input DMAs across three otherwise-idle engine queues and
        # load xt in column pieces so the first projection chains (and the
        # first attention unit, which only needs qp tile 0) start as soon as
        # possible.
        wq_sb = [sbw.tile([128, DPC], BF, tag=f"wq{i}", name=f"wq{i}") for i in range(NDIN)]
        wk_sb = [sbw.tile([128, KVW], BF, tag=f"wk{i}", name=f"wk{i}") for i in range(NDIN)]
        wv_sb = [sbw.tile([128, KVW], BF, tag=f"wv{i}", name=f"wv{i}") for i in range(NDIN)]
        xt_sb = [sbx.tile([128, p_len], BF, tag=f"xt{i}", name=f"xt{i}") for i in range(NDIN)]
        wp_sb = [sbw.tile([128, DIM], BF, tag=f"wp{i}", name=f"wp{i}") for i in range(DPC // 128)]

        # sync + gpsimd + scalar queues; scalar only carries startup loads
        # (the exp stream hasn't started yet).
        # Touch Ln and Exp immediately (before anything else lands in the
        # ScalarE queue) so the ACT table set containing both loads at kernel
        # start; the tail finalize uses Ln.
        warm = sbw.tile([1, 8], F32, tag="warm", name="warm")
        nc.vector.memset(warm[:], 1.0)
        nc.scalar.activation(warm[:], warm[:], mybir.ActivationFunctionType.Ln)
        nc.scalar.activation(warm[:], warm[:], EXP)

        # sync + gpsimd queues only: scalar must stay clear for the exp
        # stream (DMA issue costs ~0.6us each and would delay the first exp).
        dma_engines = [nc.sync, nc.gpsimd]
        dma_rr = [0]

        def dma_in(dst, src):
            eng = dma_engines[dma_rr[0] % len(dma_engines)]
            dma_rr[0] += 1
            eng.dma_start(dst, src)

        # qp-tile-0 inputs first: wq, wk and the first xt column piece.
        for i in range(NDIN):
            dma_in(wq_sb[i][:], wq_d[128 * i:128 * (i + 1), :])
            dma_in(wk_sb[i][:], wk_d[128 * i:128 * (i + 1), :])
            dma_in(xt_sb[i][:, 0:512], xt_d[128 * i:128 * (i + 1), 0:512])
        for i in range(NDIN):
            dma_in(wv_sb[i][:], wv_d[128 * i:128 * (i + 1), :])
            dma_in(xt_sb[i][:, 512:1024], xt_d[128 * i:128 * (i + 1), 512:1024])
        for t5 in range(2, p_len // 512):
            for i in range(NDIN):
                dma_in(xt_sb[i][:, 512 * t5:512 * (t5 + 1)],
                       xt_d[128 * i:128 * (i + 1), 512 * t5:512 * (t5 + 1)])
        for i in range(DPC // 128):
            dma_in(wp_sb[i][:], wp_d[128 * i:128 * (i + 1), :])

        ones_sb = sbw.tile([1, 64], BF, tag="ones", name="ones")
        nc.vector.memset(ones_sb[:], 1.0)

        LAG = 3  # PV chunks behind QK in the modulo schedule
        # ---- stage B: projection chain emitters ---------------------------
        # qT: [DPC, p_len] as row-of-128 tiles (2 heads per tile)
        # kT: per-pair tiles [128, p_len]; both halves of a shared pair are
        #     copied from the same unique-slot chain output.
        qt_sb = [sbqk.tile([128, p_len], BF, tag=f"qt{m}", name=f"qt{m}") for m in range(NPAIR)]
        kt_sb = [sbqk.tile([128, p_len], BF, tag=f"kt{m}", name=f"kt{m}") for m in range(NPAIR)]
        vplus_sb = sbqk.tile([128, NKC * S * VW], BF, tag="vplus", name="vplus")
        vp3 = vplus_sb[:].rearrange("p (kc s w) -> p kc s w", kc=NKC, s=S)

        def vplus(kc, s):
            off = (kc * S + s) * VW
            return vplus_sb[:, off:off + VW]

        nc.vector.memset(vp3[:, :, :, D:VW], 1.0)
        NT5 = p_len // 512

        def qk_chain_gen(dst, w_sb, m, t):
            ps = psb.tile([128, 512], F32, tag="psb", name="psb")
            for kd in range(NDIN):
                nc.tensor.matmul(
                    ps[:], w_sb[kd][:, 128 * m:128 * (m + 1)],
                    xt_sb[kd][:, 512 * t:512 * (t + 1)],
                    start=(kd == 0), stop=(kd == NDIN - 1))
                if kd % 2 == 1 and kd < NDIN - 1:
                    yield
            nc.vector.tensor_copy(dst[m][:, 512 * t:512 * (t + 1)], ps[:])

        def kt_chain_gen(mu, t):
            # unique-slot k chain: psum rows 0:64 = slot 2mu, 64:128 = 2mu+1
            ps = psb.tile([128, 512], F32, tag="psb", name="psb")
            for kd in range(NDIN):
                nc.tensor.matmul(
                    ps[:], wk_sb[kd][:, 128 * mu:128 * (mu + 1)],
                    xt_sb[kd][:, 512 * t:512 * (t + 1)],
                    start=(kd == 0), stop=(kd == NDIN - 1))
                if kd % 2 == 1 and kd < NDIN - 1:
                    yield
            for hf in range(2):
                s = 2 * mu + hf
                if s >= S:
                    break
                for (j, jh) in kt_dests[s]:
                    nc.vector.tensor_copy(
                        kt_sb[j][64 * jh:64 * (jh + 1), 512 * t:512 * (t + 1)],
                        ps[64 * hf:64 * (hf + 1), :])

        def v_chain_gen(rm):
            ps = psb.tile([128, 512], F32, tag="psb", name="psb")
            for kd in range(NDIN):
                nc.tensor.matmul(
                    ps[:, 0:KVW], xt_sb[kd][:, 128 * rm:128 * (rm + 1)],
                    wv_sb[kd][:],
                    start=(kd == 0), stop=(kd == NDIN - 1))
                if kd % 2 == 1 and kd < NDIN - 1:
                    yield
            nc.vector.tensor_copy(
                vp3[:, rm, :, 0:D],
                ps[:, 0:64 * S].rearrange("p (s d) -> p s d", s=S))

        def proj_gen(t, rj, ot_tiles):
            o_sb = sbo.tile([128, DIM], BF, tag="osb", name="osb")
            for e2 in range(DIM // 512):
                ps = psb.tile([128, 512], F32, tag="psb", name="psb")
                for pair in range(NPAIR):
                    nc.tensor.matmul(
                        ps[:],
                        ot_tiles[pair][:, 128 * rj:128 * (rj + 1)],
                        wp_sb[pair][:, 512 * e2:512 * (e2 + 1)],
                        start=(pair == 0), stop=(pair == NPAIR - 1))
                    if pair == 1:
                        yield
                nc.vector.tensor_copy(o_sb[:, 512 * e2:512 * (e2 + 1)], ps[:])
                yield
            row0 = (t * RPT + rj) * 128
            if t == NQT - 1:
                # tail: split across two queues to halve the final drain
                nc.sync.dma_start(out_d[row0:row0 + 64, :], o_sb[0:64, :])
                nc.gpsimd.dma_start(out_d[row0 + 64:row0 + 128, :],
                                    o_sb[64:128, :])
            else:
                # sync queue only: gpsimd is reserved for the
                # latency-sensitive normalization reciprocal DMAs.
                nc.sync.dma_start(out_d[row0:row0 + 128, :], o_sb[:])

        import heapq

        total_chunks = NQT * NPAIR * NKC
        pump_q = []   # (deadline, seq, earliest, gen)
        pump_seq = [0]

        def add_gen(deadline, earliest, gen):
            pump_seq[0] += 1
            heapq.heappush(pump_q, (deadline, pump_seq[0], earliest, gen))

        # unit 0 needs qt/kt (pair 0, t 0) before its first instruction is
        # emitted: run those chains inline (exhaust the generators).
        for _ in qk_chain_gen(qt_sb, wq_sb, 0, 0):
            pass
        for _ in kt_chain_gen(0, 0):
            pass

        for rm in range(NKC):
            add_gen(max(rm + 1, 0), 0, v_chain_gen(rm))
        for t in range(NT5):
            for m in range(NPAIR):
                dl = max((NT5 * t + m) * NKC - 2, 0)
                if not (m == 0 and t == 0):
                    add_gen(dl, 0, qk_chain_gen(qt_sb, wq_sb, m, t))
                if m < S2 // 2 and not (m == 0 and t == 0):
                    add_gen(dl, 0, kt_chain_gen(m, t))

        pump_state = {"gen": None, "dl": 0}

        def pump(g, budget=1):
            steps = 0
            while True:
                if pump_state["gen"] is None:
                    if not pump_q or pump_q[0][2] > g:
                        return
                    dl, _, _, gen = heapq.heappop(pump_q)
                    pump_state["gen"] = gen
                    pump_state["dl"] = dl
                urgent = pump_state["dl"] <= g + 2
                if steps >= budget and not urgent:
                    return
                try:
                    next(pump_state["gen"])
                    steps += 1
                except StopIteration:
                    pump_state["gen"] = None

        # ---- stage C + D: attention then projection -----------------------
        # Flat modulo-schedule over units=(t, pair): PV lags QK by LAG chunks
        # (crossing unit boundaries) so the ScalarE exp stream never starves
        # and PE never waits on the normalization chain.
        units = [(t, pair) for t in range(NQT) for pair in range(NPAIR)]

        class Unit:
            pass

        def start_unit(i):
            u = Unit()
            u.t, u.pair = units[i]
            u.qt, u.kt = qt_sb[u.pair], kt_sb[u.pair]
            u.s0, u.s1 = slot_of[2 * u.pair], slot_of[2 * u.pair + 1]
            u.pv0 = pspv.tile([128, QW], F32, tag="pv", name="pv")
            u.pv1 = pspv.tile([128, QW], F32, tag="pv", name="pv")
            u.egs = [None] * NKC
            return u

        def qk_exp(u, kc):
            sg = pssg.tile([128, 2 * QW], F32, tag="sg", name="sg")
            nc.tensor.matmul(
                sg[:, 0:QW], u.kt[0:64, 128 * kc:128 * (kc + 1)],
                u.qt[0:64, QW * u.t:QW * (u.t + 1)], start=True, stop=True)
            nc.tensor.matmul(
                sg[:, QW:2 * QW], u.kt[64:128, 128 * kc:128 * (kc + 1)],
                u.qt[64:128, QW * u.t:QW * (u.t + 1)], start=True, stop=True)
            eg = sbeg.tile([128, 2 * QW], BF, tag="eg", name="eg")
            nc.scalar.activation(eg[:], sg[:], EXP)
            u.egs[kc] = eg

        def pv_mm(u, kc):
            nc.tensor.matmul(
                u.pv0[0:VW, :], vplus(kc, u.s0), u.egs[kc][:, 0:QW],
                start=(kc == 0), stop=(kc == NKC - 1))
            nc.tensor.matmul(
                u.pv1[0:VW, :], vplus(kc, u.s1), u.egs[kc][:, QW:2 * QW],
                start=(kc == 0), stop=(kc == NKC - 1))
            u.egs[kc] = None

        def finalize_a1(u):
            # copy unnormalized outT (both heads into one [128, QW] tile) and
            # the raw sums rows to SBUF so the two pv psum banks recycle
            # quickly, and DMA-reshape the sums to [128, 8].
            u.s = sbr.tile([128, QW], F32, tag="s", name="s")
            nc.vector.tensor_copy(u.s[0:64, :], u.pv0[0:D, :])
            nc.vector.tensor_copy(u.s[64:128, :], u.pv1[0:D, :])
            sm = sbr.tile([1, 2 * QW], F32, tag="sm", name="sm")
            nc.vector.tensor_copy(sm[:, 0:QW], u.pv0[D:VW, :])
            nc.vector.tensor_copy(sm[:, QW:2 * QW], u.pv1[D:VW, :])
            u.smt = sbr.tile([128, 8], F32, tag="smt", name="smt")
            nc.gpsimd.dma_start(u.smt[:], sm[:])

        def finalize_a2(u):
            # a few chunks later (so the DVE doesn't head-of-line block on
            # the DMA): invert on DVE (exact, 128-lane-parallel), cast bf16,
            # DMA back into [1, 1024] layout for the broadcast matmuls.
            smr = sbr.tile([128, 8], F32, tag="smr", name="smr")
            nc.vector.reciprocal(smr[:], u.smt[:])
            smb = sbr.tile([128, 8], BF, tag="smb", name="smb")
            nc.vector.tensor_copy(smb[:], smr[:])
            u.rc = sbr.tile([1, 2 * QW], BF, tag="rc", name="rc")
            nc.gpsimd.dma_start(u.rc[:], smb[:])

        def finalize_b(u, ot_tiles):
            # broadcast reciprocals across partitions with two col-packed
            # (concurrent) k=1 matmuls into one PSUM bank, then scale.
            rb = psb.tile([128, QW], F32, tag="psb", name="psb")
            nc.tensor.matmul(rb[0:64, :], ones_sb[:], u.rc[:, 0:QW],
                             start=True, stop=True)
            nc.tensor.matmul(rb[64:128, :], ones_sb[:], u.rc[:, QW:2 * QW],
                             start=True, stop=True)
            ot = sbot.tile([128, QW], BF, tag=f"ot{u.pair}", name=f"ot{u.pair}")
            nc.vector.tensor_mul(ot[:], u.s[:], rb[:])
            ot_tiles[u.pair] = ot

        ot_by_t = {t: [None] * NPAIR for t in range(NQT)}
        FA2_KC = LAG + 3
        FB_KC = min(LAG + 6, NKC - 1)

        def step_prev(u, kc, g):
            # deferred post-processing of the previous unit, spread across
            # this unit's chunk stream
            if kc >= LAG and not u.fa_done:
                finalize_a1(u)
                u.fa_done = True
            elif kc >= FA2_KC and not u.fa2_done:
                finalize_a2(u)
                u.fa2_done = True
            elif kc >= FB_KC and not u.fb_done:
                finalize_b(u, ot_by_t[u.t])
                u.fb_done = True
                if u.pair == NPAIR - 1:
                    for rj in range(RPT):
                        add_gen(total_chunks + u.t, g + 1,
                                proj_gen(u.t, rj, ot_by_t[u.t]))

        def finalize_ln(u):
            # tail path (final unit): no pipeline slack for the DMA round
            # trip, but ScalarE is idle once the exp stream ends — compute
            # the reciprocal there as exp(-ln(sums)); Ln/Exp share the
            # already-loaded table set.
            u.s = sbr.tile([128, QW], F32, tag="s", name="s")
            nc.vector.tensor_copy(u.s[0:64, :], u.pv0[0:D, :])
            nc.vector.tensor_copy(u.s[64:128, :], u.pv1[0:D, :])
            sm = sbr.tile([1, 2 * QW], F32, tag="sm", name="sm")
            nc.vector.tensor_copy(sm[:, 0:QW], u.pv0[D:VW, :])
            nc.vector.tensor_copy(sm[:, QW:2 * QW], u.pv1[D:VW, :])
            lg = sbr.tile([1, 2 * QW], F32, tag="lg", name="lg")
            nc.scalar.activation(lg[:], sm[:], mybir.ActivationFunctionType.Ln)
            u.rc = sbr.tile([1, 2 * QW], BF, tag="rc", name="rc")
            nc.scalar.activation(u.rc[:], lg[:], EXP, scale=-1.0)

        def drain_prev(u, g):
            if not u.fa_done:
                finalize_ln(u)
            elif not u.fa2_done:
                finalize_a2(u)
            if not u.fb_done:
                finalize_b(u, ot_by_t[u.t])
                if u.pair == NPAIR - 1:
                    for rj in range(RPT):
                        add_gen(total_chunks + u.t, 0,
                                proj_gen(u.t, rj, ot_by_t[u.t]))

        prev = None
        cur = start_unit(0)
        for i in range(len(units)):
            cur.fa_done = cur.fa2_done = cur.fb_done = False
            for kc in range(NKC):
                g = i * NKC + kc
                qk_exp(cur, kc)
                pump(g)
                gk = kc - LAG
                if gk < 0 and prev is not None:
                    pv_mm(prev, NKC + gk)
                if prev is not None:
                    # emit the prev unit's pv->SBUF copies BEFORE this unit's
                    # first pv matmul so the PE doesn't wait on DVE for the
                    # recycled pv PSUM banks
                    step_prev(prev, kc, g)
                if gk >= 0:
                    pv_mm(cur, gk)
            if prev is not None and not prev.fb_done:
                drain_prev(prev, i * NKC + NKC - 1)
            prev, cur = cur, (start_unit(i + 1) if i + 1 < len(units) else None)
        for gk in range(NKC - LAG, NKC):
            pv_mm(prev, gk)
            pump(total_chunks)
        drain_prev(prev, total_chunks)
        g = total_chunks
        while pump_q or pump_state["gen"] is not None:
            pump(g, budget=100)
            g += 1

    if split_waits:
        _split_wide_waits(nc, max_waits=1)
    return nc


_PROGRAMS = {}


def _get_program(F):
    if F not in _PROGRAMS:
        _PROGRAMS[F] = build_program(F, P)
    return _PROGRAMS[F]


# ------------------------------------------------------------------- kernel

def make_in_maps(x, Wq, Wk, Wv, Wp, bp, cache):
    x = np.asarray(x, np.float32)
    Wq = np.asarray(Wq, np.float32)
    Wk = np.asarray(Wk, np.float32)
    Wv = np.asarray(Wv, np.float32)
    Wp = np.asarray(Wp, np.float32)
    kv_id = _kv_id(x, Wk, np.asarray(cache, np.float32))
    F, perms = _plan(kv_id)
    slot_of, kt_dests, S, S2 = _slot_map(F)

    scale = 1.0 / np.sqrt(D)
    in_maps = []
    xt_b = [np.ascontiguousarray(x[b].T).astype(BF16) for b in range(B)]
    for c in range(NCORES):
        b, half = divmod(c, 2)
        perm = perms[half]
        heads = [half * HPC + p for p in perm]
        # slot s is represented by (pair j, half jh) = kt_dests[s][0]
        slot_kv = [kv_id[heads[2 * j + jh]] for (j, jh) in
                   (kt_dests[s][0] for s in range(S))]
        wk_c = np.zeros((DIM, 64 * S2), np.float32)
        wv_c = np.zeros((DIM, 64 * S2), np.float32)
        for s, kv in enumerate(slot_kv):
            wk_c[:, 64 * s:64 * (s + 1)] = Wk[:, kv * D:(kv + 1) * D]
            wv_c[:, 64 * s:64 * (s + 1)] = Wv[:, kv * D:(kv + 1) * D]
        wq_c = np.concatenate(
            [Wq[:, h * D:(h + 1) * D] for h in heads], axis=1) * scale
        wp_c = np.concatenate(
            [Wp[h * D:(h + 1) * D, :] for h in heads], axis=0)
        in_maps.append({
            "xt": xt_b[b],
            "wq": np.ascontiguousarray(wq_c).astype(BF16),
            "wk": np.ascontiguousarray(wk_c).astype(BF16),
            "wv": np.ascontiguousarray(wv_c).astype(BF16),
            "wp": np.ascontiguousarray(wp_c).astype(BF16),
        })
    return F, in_maps


_WARMED = set()


def kernel(x, Wq, Wk, Wv, Wp, bp, cache, _trace=False):
    from concourse.bass_utils import run_bass_kernel_spmd

    F, in_maps = make_in_maps(x, Wq, Wk, Wv, Wp, bp, cache)
    nc = _get_program(F)
    if F not in _WARMED:
        # First execution on a cold NEFF has been observed racing the ACT
        # table load; run once and discard.
        run_bass_kernel_spmd(nc, in_maps, core_ids=list(range(NCORES)),
                             trace=False)
        _WARMED.add(F)
    res = run_bass_kernel_spmd(nc, in_maps, core_ids=list(range(NCORES)),
                               trace=_trace)
    bp32 = np.asarray(bp, np.float32)
    out = np.empty((B, P, DIM), np.float32)
    for b in range(B):
        out[b] = (res.results[2 * b]["out"].astype(np.float32)
                  + res.results[2 * b + 1]["out"].astype(np.float32) + bp32)
    if _trace:
        kernel.last_exec_time_ns = res.exec_time_ns
    return out


# revision 21
# speedup vs baseline: 1.0083x; 1.0021x over previous
"""Dynamic GQA attention (nn_DGQA) on 8 Trainium2 NeuronCores.

Strategy
--------
The "dynamic" part of DGQA (the query-head -> kv-head assignment, derived
from k-projection norms) is computed on the host in numpy and folded into a
host-side gather of the Wk/Wv weight columns.  The device program is a
static, uniform SPMD kernel: plain 8-head attention per core.

Sharding: core c handles batch b = c//2 and half of the query heads
(half = c%2).  Each core computes a partial output projection (contraction
over its 512 head-dims of Wp); the host sums the two partials per batch and
adds the bias.

v2 improvements over the first working version:
  * kv dedup: query heads are host-sorted by kv head so duplicated kv heads
    become adjacent pairs; the k/v projection chains compute only the unique
    kv heads ("slots"), and the PV stationary (vplus) is shared per pair.
    The program is built per sharing pattern (cached on the pattern).
  * softmax reciprocal off ScalarE: the per-unit sums row [1, 1024] is
    DMA-reshaped to [128, 8], inverted with the 128-lane-parallel DVE
    reciprocal, cast to bf16 and DMA'd back; ScalarE now runs only the exp
    stream.
  * the two per-unit reciprocal-broadcast matmuls are column-packed into a
    single PSUM bank (concurrent col tiles (0,0) / (0,64)).
  * output is written bf16 (host upcasts and sums partials) halving the
    output DMA volume; input/output DMAs are spread over several engine
    queues; xt loads are split into column pieces so the first projection
    chains start earlier.

Device kernel (per core, all matmuls bf16, fp32 accumulation):
  qT = wq.T @ xt            (wq pre-scaled by D^-0.5 on host)
  kT = wk_u.T @ xt          (unique kv slots only)
  v  = xt.T @ wv_u          (written into per-slot "vplus" tiles with a
                             ones-column per slot)
  per head pair (row-tiled QK, d=64 contraction, 2 heads concurrently):
    scoresT[key, qp] in PSUM -> exp on ScalarE -> bf16 SBUF
    PV: outT[d, qp] (+ sums row from the ones-column) accumulated over key
        chunks with the pair-shared vplus as the stationary operand
  normalization: DMA/DVE reciprocal, broadcast with col-packed k=1 matmuls,
  multiplied in on VectorE
  proj: out[row, :] partial = outT.T @ wp  (contraction over this core's
  douts), written bf16
"""

import numpy as np
import ml_dtypes

B, P, DIM, H, HKV = 4, 2048, 1024, 16, 8
D = DIM // H          # 64
NCORES = 8
HPC = H // 2          # query heads per core = 8
NPAIR = HPC // 2      # head pairs per core = 4
DPC = HPC * D         # head-dims per core = 512
VW = D + 1            # v columns per slot incl. ones column = 65

BF16 = ml_dtypes.bfloat16


# ----------------------------------------------------------------- host math

def _ratios_np(k_bhpd: np.ndarray, cache: np.ndarray) -> np.ndarray:
    """Numpy replica of the reference's _ratios (fp32, round-half-even)."""
    mags = np.sqrt((k_bhpd * k_bhpd).sum(axis=(2, 3))).sum(axis=0)
    diff = np.abs(cache - mags)
    r = np.round(diff / diff.sum() * H).astype(np.int64)
    while r.sum() > H:
        r[np.argmax(r)] -= 1
    while r.sum() < H:
        r[np.argmin(r)] += 1
    return r


def _kv_id(x: np.ndarray, Wk: np.ndarray, cache: np.ndarray) -> np.ndarray:
    k = (x.reshape(B * P, DIM).astype(np.float32) @ Wk.astype(np.float32))
    k = k.reshape(B, P, HKV, D).transpose(0, 2, 1, 3)
    r = _ratios_np(k, cache.astype(np.float32))
    return np.searchsorted(np.cumsum(r), np.arange(H), side="right")


def _plan(kv_id: np.ndarray):
    """Head ordering + kv slot plan shared by the program and the in_maps.

    Returns (F, perms) where F is the per-pair "heads share a kv head" flag
    tuple (ANDed over both halves so the SPMD program is uniform), and
    perms[half] is the local head order (sorted by kv id).
    """
    perms = []
    shared = []
    for half in range(2):
        kv = kv_id[half * HPC:(half + 1) * HPC]
        perm = sorted(range(HPC), key=lambda i: (kv[i], i))
        perms.append(perm)
        shared.append([kv[perm[2 * j]] == kv[perm[2 * j + 1]]
                       for j in range(NPAIR)])
    F = tuple(bool(shared[0][j] and shared[1][j]) for j in range(NPAIR))
    return F, perms


def _slot_map(F):
    """slot_of[local_head] and per-slot kt copy destinations [(pair, half)]."""
    slot_of = [0] * HPC
    dests = []
    for j in range(NPAIR):
        if F[j]:
            dests.append([(j, 0), (j, 1)])
            slot_of[2 * j] = slot_of[2 * j + 1] = len(dests) - 1
        else:
            dests.append([(j, 0)])
            slot_of[2 * j] = len(dests) - 1
            dests.append([(j, 1)])
            slot_of[2 * j + 1] = len(dests) - 1
    S = len(dests)
    S2 = S + (S % 2)
    return slot_of, dests, S, S2


# ----------------------------------------------------- walrus wait splitting

def _split_wide_waits(nc, max_waits=1):
    """This toolchain's walrus allows only one sync-wait per instruction;
    move extra waits onto preceding NOPs on the same engine."""
    import bass_rust
    import concourse.mybir as mybir

    n = 0
    for f in nc.m.functions:
        for blk in f.blocks:
            out = []
            changed = False
            for ins in blk.instructions:
                si = ins.sync_info
                if si is not None and si.on_wait is not None and \
                        len(si.on_wait) > max_waits:
                    waits = list(si.on_wait)
                    keep = waits[-max_waits:]
                    extra = waits[:-max_waits]
                    for j in range(0, len(extra), max_waits):
                        n += 1
                        nop = mybir.InstNoOp(
                            name=f"waitsplit-{n}", ins=[], outs=[])
                        nop.engine = ins.engine
                        nop.sync_info = bass_rust.SyncInfo(
                            on_wait=extra[j:j + max_waits], on_update=[])
                        out.append(nop)
                    ins.sync_info = bass_rust.SyncInfo(
                        on_wait=keep, on_update=list(si.on_update or []))
                    changed = True
                out.append(ins)
            if changed:
                blk.instructions = out
    return n


# ------------------------------------------------------------ device program

def build_program(F, p_len: int = P, split_waits: bool = True):
    """Build the SPMD Bass/Tile program for sharing pattern F."""
    from contextlib import ExitStack

    import concourse.bass as bass
    import concourse.tile as tile
    from concourse import mybir

    F32 = mybir.dt.float32
    BF = mybir.dt.bfloat16
    EXP = mybir.ActivationFunctionType.Exp

    slot_of, kt_dests, S, S2 = _slot_map(F)
    KVW = 64 * S2                 # wk_u / wv_u column count (padded even)

    NKC = p_len // 128            # key chunks
    NQT = max(p_len // 512, 1)    # qp tiles of width...
    QW = min(512, p_len)          # qp tile width
    NRM = p_len // 128            # output row chunks
    RPT = NRM // NQT              # row chunks per qp tile
    NDIN = DIM // 128             # contraction chunks for projections

    nc = bass.Bass("TRN2", target_bir_lowering=False, debug=False,
                   num_devices=NCORES)
    xt_d = nc.dram_tensor("xt", [DIM, p_len], BF, kind="ExternalInput").ap()
    wq_d = nc.dram_tensor("wq", [DIM, DPC], BF, kind="ExternalInput").ap()
    wk_d = nc.dram_tensor("wk", [DIM, KVW], BF, kind="ExternalInput").ap()
    wv_d = nc.dram_tensor("wv", [DIM, KVW], BF, kind="ExternalInput").ap()
    wp_d = nc.dram_tensor("wp", [DPC, DIM], BF, kind="ExternalInput").ap()
    out_d = nc.dram_tensor("out", [p_len, DIM], BF, kind="ExternalOutput").ap()

    with tile.TileContext(nc) as tc, ExitStack() as ctx:
        sbw = ctx.enter_context(tc.tile_pool(name="sbw", bufs=1))
        sbx = ctx.enter_context(tc.tile_pool(name="sbx", bufs=1))
        sbqk = ctx.enter_context(tc.tile_pool(name="sbqk", bufs=1))
        sbeg = ctx.enter_context(tc.tile_pool(name="sbeg", bufs=8))
        sbot = ctx.enter_context(tc.tile_pool(name="sbot", bufs=2))
        sbo = ctx.enter_context(tc.tile_pool(name="sbo", bufs=3))
        sbr = ctx.enter_context(tc.tile_pool(name="sbr", bufs=2))
        psb = ctx.enter_context(tc.tile_pool(name="psb", bufs=2, space="PSUM"))
        pssg = ctx.enter_context(tc.tile_pool(name="pssg", bufs=2, space="PSUM"))
        pspv = ctx.enter_context(tc.tile_pool(name="pspv", bufs=2, space="PSUM"))

        # ---- input loads -------------------------------------------------
        # Spread input DMAs across three otherwise-idle engine queues and
        # load xt in column pieces so the first projection chains (and the
        # first attention unit, which only needs qp tile 0) start as soon as
        # possible.
        wq_sb = [sbw.tile([128, DPC], BF, tag=f"wq{i}", name=f"wq{i}") for i in range(NDIN)]
        wk_sb = [sbw.tile([128, KVW], BF, tag=f"wk{i}", name=f"wk{i}") for i in range(NDIN)]
        wv_sb = [sbw.tile([128, KVW], BF, tag=f"wv{i}", name=f"wv{i}") for i in range(NDIN)]
        xt_sb = [sbx.tile([128, p_len], BF, tag=f"xt{i}", name=f"xt{i}") for i in range(NDIN)]
        wp_sb = [sbw.tile([128, DIM], BF, tag=f"wp{i}", name=f"wp{i}") for i in range(DPC // 128)]

        # sync + gpsimd + scalar queues; scalar only carries startup loads
        # (the exp stream hasn't started yet).
        # Touch Ln and Exp immediately (before anything else lands in the
        # ScalarE queue) so the ACT table set containing both loads at kernel
        # start; the tail finalize uses Ln.
        warm = sbw.tile([1, 8], F32, tag="warm", name="warm")
        nc.vector.memset(warm[:], 1.0)
        nc.scalar.activation(warm[:], warm[:], mybir.ActivationFunctionType.Ln)
        nc.scalar.activation(warm[:], warm[:], EXP)

        # sync + gpsimd queues only: scalar must stay clear for the exp
        # stream (DMA issue costs ~0.6us each and would delay the first exp).
        dma_engines = [nc.sync, nc.gpsimd]
        dma_rr = [0]

        def dma_in(dst, src):
            eng = dma_engines[dma_rr[0] % len(dma_engines)]
            dma_rr[0] += 1
            eng.dma_start(dst, src)

        # qp-tile-0 inputs first: wq, wk and the first xt column piece.
        for i in range(NDIN):
            dma_in(wq_sb[i][:], wq_d[128 * i:128 * (i + 1), :])
            dma_in(wk_sb[i][:], wk_d[128 * i:128 * (i + 1), :])
            dma_in(xt_sb[i][:, 0:512], xt_d[128 * i:128 * (i + 1), 0:512])
        for i in range(NDIN):
            dma_in(wv_sb[i][:], wv_d[128 * i:128 * (i + 1), :])
            dma_in(xt_sb[i][:, 512:1024], xt_d[128 * i:128 * (i + 1), 512:1024])
        for t5 in range(2, p_len // 512):
            for i in range(NDIN):
                dma_in(xt_sb[i][:, 512 * t5:512 * (t5 + 1)],
                       xt_d[128 * i:128 * (i + 1), 512 * t5:512 * (t5 + 1)])
        for i in range(DPC // 128):
            dma_in(wp_sb[i][:], wp_d[128 * i:128 * (i + 1), :])

        ones_sb = sbw.tile([1, 64], BF, tag="ones", name="ones")
        nc.vector.memset(ones_sb[:], 1.0)

        LAG = 4  # PV chunks behind QK in the modulo schedule
        # ---- stage B: projection chain emitters ---------------------------
        # qT: [DPC, p_len] as row-of-128 tiles (2 heads per tile)
        # kT: per-pair tiles [128, p_len]; both halves of a shared pair are
        #     copied from the same unique-slot chain output.
        qt_sb = [sbqk.tile([128, p_len], BF, tag=f"qt{m}", name=f"qt{m}") for m in range(NPAIR)]
        kt_sb = [sbqk.tile([128, p_len], BF, tag=f"kt{m}", name=f"kt{m}") for m in range(NPAIR)]
        vplus_sb = sbqk.tile([128, NKC * S * VW], BF, tag="vplus", name="vplus")
        vp3 = vplus_sb[:].rearrange("p (kc s w) -> p kc s w", kc=NKC, s=S)

        def vplus(kc, s):
            off = (kc * S + s) * VW
            return vplus_sb[:, off:off + VW]

        nc.vector.memset(vp3[:, :, :, D:VW], 1.0)
        NT5 = p_len // 512

        def qk_chain_gen(dst, w_sb, m, t):
            ps = psb.tile([128, 512], F32, tag="psb", name="psb")
            for kd in range(NDIN):
                nc.tensor.matmul(
                    ps[:], w_sb[kd][:, 128 * m:128 * (m + 1)],
                    xt_sb[kd][:, 512 * t:512 * (t + 1)],
                    start=(kd == 0), stop=(kd == NDIN - 1))
                if kd % 2 == 1 and kd < NDIN - 1:
                    yield
            nc.vector.tensor_copy(dst[m][:, 512 * t:512 * (t + 1)], ps[:])

        def kt_chain_gen(mu, t):
            # unique-slot k chain: psum rows 0:64 = slot 2mu, 64:128 = 2mu+1
            ps = psb.tile([128, 512], F32, tag="psb", name="psb")
            for kd in range(NDIN):
                nc.tensor.matmul(
                    ps[:], wk_sb[kd][:, 128 * mu:128 * (mu + 1)],
                    xt_sb[kd][:, 512 * t:512 * (t + 1)],
                    start=(kd == 0), stop=(kd == NDIN - 1))
                if kd % 2 == 1 and kd < NDIN - 1:
                    yield
            for hf in range(2):
                s = 2 * mu + hf
                if s >= S:
                    break
                for (j, jh) in kt_dests[s]:
                    nc.vector.tensor_copy(
                        kt_sb[j][64 * jh:64 * (jh + 1), 512 * t:512 * (t + 1)],
                        ps[64 * hf:64 * (hf + 1), :])

        def v_chain_gen(rm):
            ps = psb.tile([128, 512], F32, tag="psb", name="psb")
            for kd in range(NDIN):
                nc.tensor.matmul(
                    ps[:, 0:KVW], xt_sb[kd][:, 128 * rm:128 * (rm + 1)],
                    wv_sb[kd][:],
                    start=(kd == 0), stop=(kd == NDIN - 1))
                if kd % 2 == 1 and kd < NDIN - 1:
                    yield
            nc.vector.tensor_copy(
                vp3[:, rm, :, 0:D],
                ps[:, 0:64 * S].rearrange("p (s d) -> p s d", s=S))

        def proj_gen(t, rj, ot_tiles):
            o_sb = sbo.tile([128, DIM], BF, tag="osb", name="osb")
            for e2 in range(DIM // 512):
                ps = psb.tile([128, 512], F32, tag="psb", name="psb")
                for pair in range(NPAIR):
                    nc.tensor.matmul(
                        ps[:],
                        ot_tiles[pair][:, 128 * rj:128 * (rj + 1)],
                        wp_sb[pair][:, 512 * e2:512 * (e2 + 1)],
                        start=(pair == 0), stop=(pair == NPAIR - 1))
                    if pair == 1:
                        yield
                nc.vector.tensor_copy(o_sb[:, 512 * e2:512 * (e2 + 1)], ps[:])
                yield
            row0 = (t * RPT + rj) * 128
            if t == NQT - 1:
                # tail: split across two queues to halve the final drain
                nc.sync.dma_start(out_d[row0:row0 + 64, :], o_sb[0:64, :])
                nc.gpsimd.dma_start(out_d[row0 + 64:row0 + 128, :],
                                    o_sb[64:128, :])
            else:
                # sync queue only: gpsimd is reserved for the
                # latency-sensitive normalization reciprocal DMAs.
                nc.sync.dma_start(out_d[row0:row0 + 128, :], o_sb[:])

        import heapq

        total_chunks = NQT * NPAIR * NKC
        pump_q = []   # (deadline, seq, earliest, gen)
        pump_seq = [0]

        def add_gen(deadline, earliest, gen):
            pump_seq[0] += 1
            heapq.heappush(pump_q, (deadline, pump_seq[0], earliest, gen))

        # unit 0 needs qt/kt (pair 0, t 0) before its first instruction is
        # emitted: run those chains inline (exhaust the generators).
        for _ in qk_chain_gen(qt_sb, wq_sb, 0, 0):
            pass
        for _ in kt_chain_gen(0, 0):
            pass

        for rm in range(NKC):
            add_gen(max(rm + 1, 0), 0, v_chain_gen(rm))
        for t in range(NT5):
            for m in range(NPAIR):
                dl = max((NT5 * t + m) * NKC - 2, 0)
                if not (m == 0 and t == 0):
                    add_gen(dl, 0, qk_chain_gen(qt_sb, wq_sb, m, t))
                if m < S2 // 2 and not (m == 0 and t == 0):
                    add_gen(dl, 0, kt_chain_gen(m, t))

        pump_state = {"gen": None, "dl": 0}

        def pump(g, budget=1):
            steps = 0
            while True:
                if pump_state["gen"] is None:
                    if not pump_q or pump_q[0][2] > g:
                        return
                    dl, _, _, gen = heapq.heappop(pump_q)
                    pump_state["gen"] = gen
                    pump_state["dl"] = dl
                urgent = pump_state["dl"] <= g + 2
                if steps >= budget and not urgent:
                    return
                try:
                    next(pump_state["gen"])
                    steps += 1
                except StopIteration:
                    pump_state["gen"] = None

        # ---- stage C + D: attention then projection -----------------------
        # Flat modulo-schedule over units=(t, pair): PV lags QK by LAG chunks
        # (crossing unit boundaries) so the ScalarE exp stream never starves
        # and PE never waits on the normalization chain.
        units = [(t, pair) for t in range(NQT) for pair in range(NPAIR)]

        class Unit:
            pass

        def start_unit(i):
            u = Unit()
            u.t, u.pair = units[i]
            u.qt, u.kt = qt_sb[u.pair], kt_sb[u.pair]
            u.s0, u.s1 = slot_of[2 * u.pair], slot_of[2 * u.pair + 1]
            u.pv0 = pspv.tile([128, QW], F32, tag="pv", name="pv")
            u.pv1 = pspv.tile([128, QW], F32, tag="pv", name="pv")
            u.egs = [None] * NKC
            return u

        def qk_exp(u, kc):
            sg = pssg.tile([128, 2 * QW], F32, tag="sg", name="sg")
            nc.tensor.matmul(
                sg[:, 0:QW], u.kt[0:64, 128 * kc:128 * (kc + 1)],
                u.qt[0:64, QW * u.t:QW * (u.t + 1)], start=True, stop=True)
            nc.tensor.matmul(
                sg[:, QW:2 * QW], u.kt[64:128, 128 * kc:128 * (kc + 1)],
                u.qt[64:128, QW * u.t:QW * (u.t + 1)], start=True, stop=True)
            eg = sbeg.tile([128, 2 * QW], BF, tag="eg", name="eg")
            nc.scalar.activation(eg[:], sg[:], EXP)
            u.egs[kc] = eg

        def pv_mm(u, kc):
            nc.tensor.matmul(
                u.pv0[0:VW, :], vplus(kc, u.s0), u.egs[kc][:, 0:QW],
                start=(kc == 0), stop=(kc == NKC - 1))
            nc.tensor.matmul(
                u.pv1[0:VW, :], vplus(kc, u.s1), u.egs[kc][:, QW:2 * QW],
                start=(kc == 0), stop=(kc == NKC - 1))
            u.egs[kc] = None

        def finalize_a1(u):
            # copy unnormalized outT (both heads into one [128, QW] tile) and
            # the raw sums rows to SBUF so the two pv psum banks recycle
            # quickly, and DMA-reshape the sums to [128, 8].
            u.s = sbr.tile([128, QW], F32, tag="s", name="s")
            nc.vector.tensor_copy(u.s[0:64, :], u.pv0[0:D, :])
            nc.vector.tensor_copy(u.s[64:128, :], u.pv1[0:D, :])
            sm = sbr.tile([1, 2 * QW], F32, tag="sm", name="sm")
            nc.vector.tensor_copy(sm[:, 0:QW], u.pv0[D:VW, :])
            nc.vector.tensor_copy(sm[:, QW:2 * QW], u.pv1[D:VW, :])
            u.smt = sbr.tile([128, 8], F32, tag="smt", name="smt")
            nc.gpsimd.dma_start(u.smt[:], sm[:])

        def finalize_a2(u):
            # a few chunks later (so the DVE doesn't head-of-line block on
            # the DMA): invert on DVE (exact, 128-lane-parallel), cast bf16,
            # DMA back into [1, 1024] layout for the broadcast matmuls.
            smr = sbr.tile([128, 8], F32, tag="smr", name="smr")
            nc.vector.reciprocal(smr[:], u.smt[:])
            smb = sbr.tile([128, 8], BF, tag="smb", name="smb")
            nc.vector.tensor_copy(smb[:], smr[:])
            u.rc = sbr.tile([1, 2 * QW], BF, tag="rc", name="rc")
            nc.gpsimd.dma_start(u.rc[:], smb[:])

        def finalize_b(u, ot_tiles):
            # broadcast reciprocals across partitions with two col-packed
            # (concurrent) k=1 matmuls into one PSUM bank, then scale.
            rb = psb.tile([128, QW], F32, tag="psb", name="psb")
            nc.tensor.matmul(rb[0:64, :], ones_sb[:], u.rc[:, 0:QW],
                             start=True, stop=True)
            nc.tensor.matmul(rb[64:128, :], ones_sb[:], u.rc[:, QW:2 * QW],
                             start=True, stop=True)
            ot = sbot.tile([128, QW], BF, tag=f"ot{u.pair}", name=f"ot{u.pair}")
            nc.vector.tensor_mul(ot[:], u.s[:], rb[:])
            ot_tiles[u.pair] = ot

        ot_by_t = {t: [None] * NPAIR for t in range(NQT)}
        FA2_KC = LAG + 3
        FB_KC = min(LAG + 6, NKC - 1)

        def step_prev(u, kc, g):
            # deferred post-processing of the previous unit, spread across
            # this unit's chunk stream
            if kc >= LAG and not u.fa_done:
                finalize_a1(u)
                u.fa_done = True
            elif kc >= FA2_KC and not u.fa2_done:
                finalize_a2(u)
                u.fa2_done = True
            elif kc >= FB_KC and not u.fb_done:
                finalize_b(u, ot_by_t[u.t])
                u.fb_done = True
                if u.pair == NPAIR - 1:
                    for rj in range(RPT):
                        add_gen(total_chunks + u.t, g + 1,
                                proj_gen(u.t, rj, ot_by_t[u.t]))

        def finalize_ln(u):
            # tail path (final unit): no pipeline slack for the DMA round
            # trip, but ScalarE is idle once the exp stream ends — compute
            # the reciprocal there as exp(-ln(sums)); Ln/Exp share the
            # already-loaded table set.
            u.s = sbr.tile([128, QW], F32, tag="s", name="s")
            nc.vector.tensor_copy(u.s[0:64, :], u.pv0[0:D, :])
            nc.vector.tensor_copy(u.s[64:128, :], u.pv1[0:D, :])
            sm = sbr.tile([1, 2 * QW], F32, tag="sm", name="sm")
            nc.vector.tensor_copy(sm[:, 0:QW], u.pv0[D:VW, :])
            nc.vector.tensor_copy(sm[:, QW:2 * QW], u.pv1[D:VW, :])
            lg = sbr.tile([1, 2 * QW], F32, tag="lg", name="lg")
            nc.scalar.activation(lg[:], sm[:], mybir.ActivationFunctionType.Ln)
            u.rc = sbr.tile([1, 2 * QW], BF, tag="rc", name="rc")
            nc.scalar.activation(u.rc[:], lg[:], EXP, scale=-1.0)

        def drain_prev(u, g):
            if not u.fa_done:
                finalize_ln(u)
            elif not u.fa2_done:
                finalize_a2(u)
            if not u.fb_done:
                finalize_b(u, ot_by_t[u.t])
                if u.pair == NPAIR - 1:
                    for rj in range(RPT):
                        add_gen(total_chunks + u.t, 0,
                                proj_gen(u.t, rj, ot_by_t[u.t]))

        prev = None
        cur = start_unit(0)
        for i in range(len(units)):
            cur.fa_done = cur.fa2_done = cur.fb_done = False
            for kc in range(NKC):
                g = i * NKC + kc
                qk_exp(cur, kc)
                pump(g)
                gk = kc - LAG
                if gk < 0 and prev is not None:
                    pv_mm(prev, NKC + gk)
                if prev is not None:
                    # emit the prev unit's pv->SBUF copies BEFORE this unit's
                    # first pv matmul so the PE doesn't wait on DVE for the
                    # recycled pv PSUM banks
                    step_prev(prev, kc, g)
                if gk >= 0:
                    pv_mm(cur, gk)
            if prev is not None and not prev.fb_done:
                drain_prev(prev, i * NKC + NKC - 1)
            prev, cur = cur, (start_unit(i + 1) if i + 1 < len(units) else None)
        for gk in range(NKC - LAG, NKC):
            pv_mm(prev, gk)
            pump(total_chunks)
        drain_prev(prev, total_chunks)
        g = total_chunks
        while pump_q or pump_state["gen"] is not None:
            pump(g, budget=100)
            g += 1

    if split_waits:
        _split_wide_waits(nc, max_waits=1)
    return nc


_PROGRAMS = {}


def _get_program(F):
    if F not in _PROGRAMS:
        _PROGRAMS[F] = build_program(F, P)
    return _PROGRAMS[F]


# ------------------------------------------------------------------- kernel

def make_in_maps(x, Wq, Wk, Wv, Wp, bp, cache):
    x = np.asarray(x, np.float32)
    Wq = np.asarray(Wq, np.float32)
    Wk = np.asarray(Wk, np.float32)
    Wv = np.asarray(Wv, np.float32)
    Wp = np.asarray(Wp, np.float32)
    kv_id = _kv_id(x, Wk, np.asarray(cache, np.float32))
    F, perms = _plan(kv_id)
    slot_of, kt_dests, S, S2 = _slot_map(F)

    scale = 1.0 / np.sqrt(D)
    in_maps = []
    xt_b = [np.ascontiguousarray(x[b].T).astype(BF16) for b in range(B)]
    for c in range(NCORES):
        b, half = divmod(c, 2)
        perm = perms[half]
        heads = [half * HPC + p for p in perm]
        # slot s is represented by (pair j, half jh) = kt_dests[s][0]
        slot_kv = [kv_id[heads[2 * j + jh]] for (j, jh) in
                   (kt_dests[s][0] for s in range(S))]
        wk_c = np.zeros((DIM, 64 * S2), np.float32)
        wv_c = np.zeros((DIM, 64 * S2), np.float32)
        for s, kv in enumerate(slot_kv):
            wk_c[:, 64 * s:64 * (s + 1)] = Wk[:, kv * D:(kv + 1) * D]
            wv_c[:, 64 * s:64 * (s + 1)] = Wv[:, kv * D:(kv + 1) * D]
        wq_c = np.concatenate(
            [Wq[:, h * D:(h + 1) * D] for h in heads], axis=1) * scale
        wp_c = np.concatenate(
            [Wp[h * D:(h + 1) * D, :] for h in heads], axis=0)
        in_maps.append({
            "xt": xt_b[b],
            "wq": np.ascontiguousarray(wq_c).astype(BF16),
            "wk": np.ascontiguousarray(wk_c).astype(BF16),
            "wv": np.ascontiguousarray(wv_c).astype(BF16),
            "wp": np.ascontiguousarray(wp_c).astype(BF16),
        })
    return F, in_maps


_WARMED = set()


def kernel(x, Wq, Wk, Wv, Wp, bp, cache, _trace=False):
    from concourse.bass_utils import run_bass_kernel_spmd

    F, in_maps = make_in_maps(x, Wq, Wk, Wv, Wp, bp, cache)
    nc = _get_program(F)
    if F not in _WARMED:
        # First execution on a cold NEFF has been observed racing the ACT
        # table load; run once and discard.
        run_bass_kernel_spmd(nc, in_maps, core_ids=list(range(NCORES)),
                             trace=False)
        _WARMED.add(F)
    res = run_bass_kernel_spmd(nc, in_maps, core_ids=list(range(NCORES)),
                               trace=_trace)
    bp32 = np.asarray(bp, np.float32)
    out = np.empty((B, P, DIM), np.float32)
    for b in range(B):
        out[b] = (res.results[2 * b]["out"].astype(np.float32)
                  + res.results[2 * b + 1]["out"].astype(np.float32) + bp32)
    if _trace:
        kernel.last_exec_time_ns = res.exec_time_ns
    return out
